# revision 68
# baseline (speedup 1.0000x reference)
"""Causal GQA attention (B=2,T=2048,D=1024,H=16,KV=4) on 8 trn2 cores.

Sharding: core = b*4 + g  (batch b, kv-group g).  Each core computes the
4 query heads of its group for its batch plus the row-parallel partial of
the output projection; the host sums the 4 partials per batch.

v4 design notes:
- single interleaved emission: projections (kv/q per 512-block) are cut
  into small steps and drained from a "PE filler queue" inside the
  attention kt-loop, so the PE never idles while the scalar engine
  (exp, the real bottleneck) catches up, and HAM stays warm.
- scalar engine runs ONLY exp; psum evictions on vector, sbuf-only rope
  muls/adds on gpsimd.
- all bulk inputs arrive host-pre-tiled so each is ONE dma with 2-8KB
  contiguous per-partition lines (the dynamic dma ring is FIFO; many
  small loads starved the early kernel for ~40us in v3).
- softmax normalization with no DMA and no gpsimd extended-inst (both
  bit us): full-height ctx eviction to sbuf f32 (row 64 = l), DVE
  reciprocal at partition 64, K=1 matmul broadcast with a base-64 ones
  vector, stt multiply.  ctx psum frees after ~1us so the next pair's
  attnv never waits.
- psum: scores ring 2x[128,2,512] (4 banks) + ctx_e/ctx_o (2) + shared
  proj/oproj/broadcast ring 2x[128,512] (2) = exactly 8 banks.
"""

import os
import numpy as np
import ml_dtypes

import concourse.bass as bass
import concourse.tile as tile
import concourse.mybir as mybir
from concourse import bacc
from concourse.bass_utils import run_bass_kernel_spmd
from concourse.masks import make_identity

F32 = mybir.dt.float32
F16 = mybir.dt.float16
BF16 = mybir.dt.bfloat16
AF = mybir.ActivationFunctionType
MUL = mybir.AluOpType.mult

B, T, C, HEADS, KVH, HD = 2, 2048, 1024, 16, 4, 64
G = HEADS // KVH          # 4 query heads per kv group
DG = G * HD               # 256 columns per group
NCORES = 8
SCALE = 1.0 / 8.0         # 1/sqrt(HD)
NT = T // 512             # 4 q-blocks of 512
NKT = T // 128            # 16 k-tiles of 128

_CACHE = {}
LAST_EXEC_NS = None


def _install_trace_hook():
    import sys, types
    try:
        import antenv.axon_hooks  # noqa: F401
        return
    except ImportError:
        pass
    try:
        from trn_agent_boot.trn_boot import _ntff_profile_via_ctypes
        hook = _ntff_profile_via_ctypes('/opt/axon/libaxon_pjrt.so')
    except Exception:
        hook = None
    mod = types.ModuleType('antenv.axon_hooks')
    mod.get_axon_ntff_profile_hook = lambda: hook
    mod.set_axon_ntff_profile_hook = lambda h: None
    sys.modules['antenv.axon_hooks'] = mod


class Filler:
    """Queue of small PE-work emissions drained inside attention loops."""

    def __init__(self):
        self.steps = []
        self.pos = 0
        self.marks = {}
        self.fallback = None
        self.fallback_budget = 0

    def add_unit(self, name, steps):
        self.steps.extend(steps)
        self.marks[name] = len(self.steps)

    def add_units(self, name, *lists):
        """Round-robin interleave step lists (avoids PE FIFO head-of-line
        blocks when one unit's chain waits on another engine)."""
        out = []
        for i in range(max(len(a) for a in lists)):
            for a in lists:
                if i < len(a):
                    out.append(a[i])
        self.steps.extend(out)
        self.marks[name] = len(self.steps)

    def drain(self, n):
        end = min(self.pos + n, len(self.steps))
        if self.pos >= end and self.fallback and self.fallback_budget > 0:
            # filler dry: emit one warm-up matmul so the PE clock gate
            # (HAM) never sees an idle window and re-throttles to 1.2GHz
            self.fallback_budget -= 1
            self.fallback()
            return
        while self.pos < end:
            self.steps[self.pos]()
            self.pos += 1

    def ensure(self, name):
        end = self.marks[name]
        while self.pos < end:
            self.steps[self.pos]()
            self.pos += 1

    def drain_all(self):
        while self.pos < len(self.steps):
            self.steps[self.pos]()
            self.pos += 1


def _build(debug=False):
    nc = bacc.Bacc("TRN2", target_bir_lowering=False, debug=debug)

    xtl_d = nc.dram_tensor("xtl", [NT, 128, 8, 512], BF16,
                           kind="ExternalInput")
    sctl_d = nc.dram_tensor("sctl", [NT, 128, 2, 512], BF16,
                            kind="ExternalInput")
    maskb_d = nc.dram_tensor("maskb", [16, 128], F32, kind="ExternalInput")
    wqtl_d = nc.dram_tensor("wqtl", [2, 128, 8, 128], BF16,
                            kind="ExternalInput")
    wkvtl_d = nc.dram_tensor("wkvtl", [128, 8, 128], BF16,
                             kind="ExternalInput")
    wotl_d = nc.dram_tensor("wotl", [128, 2, C], BF16, kind="ExternalInput")
    rt_d = nc.dram_tensor("rt", [128, 128], BF16, kind="ExternalInput")
    tri_d = nc.dram_tensor("tri", [128, 128], BF16, kind="ExternalInput")
    y_d = nc.dram_tensor("ytl", [NT, 8, 128, 512], F16, kind="ExternalOutput")
    if debug:
        khat_dump = nc.dram_tensor("khat_dump", [128, T], BF16,
                                   kind="ExternalOutput")
        qhat_dump = [nc.dram_tensor(f"qhat_dump{m}", [128, T], BF16,
                                    kind="ExternalOutput") for m in range(2)]
        vp_dump = nc.dram_tensor("vp_dump", [128, NKT * (HD + 1)], BF16,
                                 kind="ExternalOutput")
        ctxn_dump = [nc.dram_tensor(f"ctxn_dump{m}", [128, T], BF16,
                                    kind="ExternalOutput") for m in range(2)]
        ctxe_dump = nc.dram_tensor("ctxe_dump", [65, 512], F32,
                                   kind="ExternalOutput")
        ctxo_dump = nc.dram_tensor("ctxo_dump", [65, 512], F32,
                                   kind="ExternalOutput")
        rrb_dump = nc.dram_tensor("rrb_dump", [64, 2 * 512], F32,
                                  kind="ExternalOutput")

    with tile.TileContext(nc) as tc:
        with (
            tc.tile_pool(name="persist", bufs=1) as persist,
            tc.tile_pool(name="stage", bufs=3) as stage,
            tc.tile_pool(name="ps", bufs=2, space="PSUM") as ps,
            tc.tile_pool(name="pc", bufs=1, space="PSUM") as pc,
            tc.tile_pool(name="py", bufs=2, space="PSUM") as py,
        ):
            # pin psum tag creation order: sp ring (4 banks), ctx (2), yp (2)
            heat_sp = ps.tile([128, 2, 512], F32, tag="sp", name="heat_sp")
            nc.vector.memset(heat_sp[0:1, 0:1, 0:8], 0.0)
            heat_ce = pc.tile([65, 512], F32, tag="ctxe", name="heat_ce")
            nc.vector.memset(heat_ce[0:1, 0:8], 0.0)
            heat_co = pc.tile([65, 512], F32, tag="ctxo", name="heat_co")
            nc.vector.memset(heat_co[0:1, 0:8], 0.0)

            # ---- small constants (dma order = priority order) ----
            rt_sb = persist.tile([128, 128], BF16, tag="rt")
            nc.sync.dma_start(out=rt_sb[:], in_=rt_d[:, :])
            mb_sb = persist.tile([16, 128], F32, tag="mb")
            nc.sync.dma_start(out=mb_sb[:], in_=maskb_d[:, :])
            tri = persist.tile([128, 128], BF16, tag="tri")
            nc.sync.dma_start(out=tri[:], in_=tri_d[:, :])

            # persistent activation tensors
            sc2t = persist.tile([128, 2, T], BF16, tag="sc2t")
            xtbf = persist.tile([128, 8, T], BF16, tag="xtbf")
            wkvbf = persist.tile([128, 8, 128], BF16, tag="wkvbf")
            wqbf = [persist.tile([128, 8, 128], BF16, tag=f"wqbf{m}",
                                 name=f"wqbf{m}") for m in range(2)]
            wobf = persist.tile([128, 2, C], BF16, tag="wobf")
            khat2 = persist.tile([128, T], BF16, tag="khat2")
            qhat2 = [persist.tile([128, T], BF16, tag=f"qhat{m}",
                                 name=f"qhat{m}") for m in range(2)]
            vp = persist.tile([128, NKT, HD + 1], BF16, tag="vp")
            ctxn = [persist.tile([128, T], BF16, tag=f"ctxn{mi}",
                                 name=f"ctxn{mi}") for mi in range(2)]
            kmask01 = persist.tile([128, NKT], F32, tag="kmask01")
            ones64 = persist.tile([1, 64], BF16, tag="ones64")
            nc.vector.memset(ones64[:, :], 1.0)
            if debug:
                dbg_ctxe = persist.tile([65, 512], F32, tag="dbg_ctxe")
                dbg_ctxo = persist.tile([65, 512], F32, tag="dbg_ctxo")
                dbg_rrb = persist.tile([64, 2, 512], F32, tag="dbg_rrb")

            def dma_sc(tb):
                ts_ = slice(tb * 512, (tb + 1) * 512)
                nc.sync.dma_start(out=sc2t[:, :, ts_], in_=sctl_d[tb])

            def dma_x(tb, eng=None):
                ts_ = slice(tb * 512, (tb + 1) * 512)
                (eng or nc.sync).dma_start(out=xtbf[:, :, ts_],
                                           in_=xtl_d[tb])

            # split the early bulk loads across both HWDGE rings (each ring
            # sustains only ~120GB/s): scalar ring is idle until first exp.
            # x0 goes in quarter-chunks so kv-proj mms start streaming after
            # the first 256KB.
            nc.sync.dma_start(out=wkvbf[:, :, :], in_=wkvtl_d[:, :, :])
            for c, eng in ((0, nc.scalar), (1, nc.sync),
                           (2, nc.scalar), (3, nc.sync)):
                eng.dma_start(out=xtbf[:, c * 2:(c + 1) * 2, 0:512],
                              in_=xtl_d[0, :, c * 2:(c + 1) * 2, :])
            nc.scalar.dma_start(out=wqbf[0][:, :, :], in_=wqtl_d[0])
            dma_sc(0)
            nc.sync.dma_start(out=wqbf[1][:, :, :], in_=wqtl_d[1])

            # identities
            id16 = persist.tile([16, 16], F32, tag="id16")
            make_identity(nc, id16[:])
            id64b = persist.tile([128, 64], BF16, tag="id64b")
            make_identity(nc, id64b[0:64, :])
            nc.sync.dma_start(out=id64b[64:128, :], in_=id64b[0:64, :])

            # padding mask -> per-k 0/1 column layout [128, NKT]
            mt = py.tile([128, 512], F32, tag="yp", name="mt")
            nc.tensor.transpose(mt[:, 0:16], mb_sb[:], id16[:])
            nc.vector.tensor_scalar(
                out=kmask01[:], in0=mt[:, 0:16], scalar1=0.0, scalar2=None,
                op0=mybir.AluOpType.is_gt)

            # ================= emission units =================
            def unit_kv(tb, early=False):
                ts_ = slice(tb * 512, (tb + 1) * 512)
                ve = nc.vector if early else nc.gpsimd
                steps = []
                st = {}

                def pre():
                    # prefetch next tb's bulk inputs (x1 early via the idle
                    # scalar ring; later x's via sync once bulk is done)
                    if tb + 1 < NT:
                        dma_x(tb + 1, eng=nc.scalar if tb == 0 else None)
                        dma_sc(tb + 1)
                    st['pkv'] = py.tile([128, 512], F32, tag="yp",
                                        name="pkv")
                    for ct in (0, 1):
                        nc.tensor.matmul(st['pkv'][:], wkvbf[:, ct, :],
                                         xtbf[:, ct, ts_],
                                         start=(ct == 0), stop=False)
                steps.append(pre)

                def mk_mm(c0):
                    def f():
                        for ct in (c0, c0 + 1):
                            nc.tensor.matmul(st['pkv'][:], wkvbf[:, ct, :],
                                             xtbf[:, ct, ts_],
                                             start=False, stop=(ct == 7))
                    return f
                for c0 in range(2, 8, 2):
                    steps.append(mk_mm(c0))

                def evict():
                    # one DVE copy evicts both k (0:64) and v (64:128) halves
                    st['kvb'] = stage.tile([128, 512], BF16, tag="kvbbf",
                                           name="kvb")
                    nc.vector.tensor_copy(out=st['kvb'][:],
                                          in_=st['pkv'][:, :])
                steps.append(evict)

                def krope():
                    st['krot'] = py.tile([128, 512], F32, tag="yp",
                                         name="krot")
                    nc.tensor.matmul(st['krot'][0:64, :], rt_sb[0:64, 0:64],
                                     st['kvb'][0:64, :],
                                     start=True, stop=True)
                    t1k = stage.tile([64, 512], BF16, tag="t1k")
                    ve.tensor_mul(t1k[:], st['kvb'][0:64, :],
                                  sc2t[0:64, 1, ts_])
                    t2k = stage.tile([64, 512], BF16, tag="t2k")
                    nc.vector.tensor_mul(t2k[:], st['krot'][0:64, :],
                                         sc2t[0:64, 0, ts_])
                    ve.tensor_add(khat2[0:64, ts_], t1k[:], t2k[:])
                steps.append(krope)

                def kdup():
                    # duplicate khat to partitions 64:128 via identity
                    # matmul (any DMA here lands behind bulk loads and
                    # stalls the first scores by ~10us)
                    khps = py.tile([128, 512], F32, tag="yp", name="khps")
                    nc.tensor.matmul(khps[64:128, :], id64b[0:64, :],
                                     khat2[0:64, ts_],
                                     start=True, stop=True)
                    nc.vector.tensor_copy(out=khat2[64:128, ts_],
                                          in_=khps[64:128, :])
                steps.append(kdup)

                def vtrans():
                    st['vt'] = py.tile([128, 4, HD], BF16, tag="yp",
                                       name="vt")
                    for k4 in range(4):
                        nc.tensor.transpose(
                            st['vt'][:, k4, :],
                            st['kvb'][64:128, k4 * 128:(k4 + 1) * 128],
                            id64b[64:128, :])
                steps.append(vtrans)

                def mk_vp(k4):
                    def f():
                        kt = tb * 4 + k4
                        nc.vector.tensor_scalar(
                            out=vp[:, kt, 0:HD], in0=st['vt'][:, k4, :],
                            scalar1=kmask01[:, kt:kt + 1], scalar2=None,
                            op0=MUL)
                        nc.vector.tensor_copy(
                            out=vp[:, kt, HD:HD + 1],
                            in_=kmask01[:, kt:kt + 1])
                    return f
                for k4 in range(4):
                    steps.append(mk_vp(k4))
                return steps

            def unit_q(tb, m, early=False):
                ts_ = slice(tb * 512, (tb + 1) * 512)
                ve = nc.vector if early else nc.gpsimd
                steps = []
                st = {}

                def mk_mm(c0):
                    def f():
                        if c0 == 0:
                            st['pq'] = py.tile([128, 512], F32, tag="yp",
                                               name="pq")
                        for ct in (c0, c0 + 1):
                            nc.tensor.matmul(st['pq'][:], wqbf[m][:, ct, :],
                                             xtbf[:, ct, ts_],
                                             start=(ct == 0), stop=(ct == 7))
                    return f
                for c0 in range(0, 8, 2):
                    steps.append(mk_mm(c0))

                def evict():
                    st['qb'] = stage.tile([128, 512], BF16, tag="qbbf",
                                          name="qb")
                    nc.vector.tensor_copy(out=st['qb'][:], in_=st['pq'][:])
                steps.append(evict)

                def qrope():
                    if tb == 1 and m == 0:
                        # defer wo load until mid-kernel (first use: oproj(0))
                        nc.sync.dma_start(out=wobf[:, :, :],
                                          in_=wotl_d[:, :, :])
                    st['prot'] = py.tile([128, 512], F32, tag="yp",
                                         name="prot")
                    nc.tensor.matmul(st['prot'][:], rt_sb[:], st['qb'][:],
                                     start=True, stop=True)
                    t1 = stage.tile([128, 512], BF16, tag="t1")
                    ve.tensor_mul(t1[:], st['qb'][:],
                                  sc2t[:, 1, ts_])
                    t2 = stage.tile([128, 512], BF16, tag="t2")
                    nc.vector.tensor_mul(t2[:], st['prot'][:],
                                         sc2t[:, 0, ts_])
                    ve.tensor_add(qhat2[m][:, ts_], t1[:], t2[:])
                steps.append(qrope)
                return steps

            def unit_oproj(qb, last=False):
                qs = slice(qb * 512, (qb + 1) * 512)
                steps = []

                def mk_et(et):
                    def f():
                        yp = py.tile([128, 512], F32, tag="yp", name="yp")
                        ec = slice(et * 128, (et + 1) * 128)
                        for mi in range(2):
                            nc.tensor.matmul(yp[:], wobf[:, mi, ec],
                                             ctxn[mi][:, qs],
                                             start=(mi == 0), stop=(mi == 1))
                        ysb = stage.tile([128, 512], F16, tag="ysb", bufs=4)
                        nc.vector.tensor_copy(out=ysb[:], in_=yp[:])
                        nc.sync.dma_start(out=y_d[qb, et], in_=ysb[:])
                    return f
                for et in range(8):
                    steps.append(mk_et(et))
                return steps

            filler = Filler()

            def warm_mm():
                junk = py.tile([128, 512], F32, tag="yp", name="junk")
                nc.tensor.matmul(junk[:], tri[:], khat2[:, 0:512],
                                 start=True, stop=True)
            filler.fallback = warm_mm
            # small budget: fires only when the filler is dry (late, ACT-
            # paced q-blocks and the final norm chain), keeping the PE
            # clock warm into the tail oproj
            filler.fallback_budget = 20

            # filler drain rate per kt, by q-block (front-load proj work;
            # late q-blocks are ACT-bound with little filler left)
            # (first, second) drain counts per 2-kt burst; qb3 is ACT-paced,
            # so keep PE filler light there
            DRAINS = {0: (3, 3), 1: (3, 3), 2: (2, 2), 3: (1, 0)}

            # ---- attention pair (qb, m) ----
            def attn(qb, m):
                qlo = qb * 512
                nkt = 4 * (qb + 1)
                ctx_e = pc.tile([65, 512], F32, tag="ctxe", name="ctxe")
                ctx_o = pc.tile([65, 512], F32, tag="ctxo", name="ctxo")
                pbfs = [None] * nkt
                offs = [None] * nkt

                def scores(kt):
                    o = (kt - 4 * qb) * 128 if kt >= 4 * qb else 0
                    offs[kt] = o
                    kc = slice(kt * 128, (kt + 1) * 128)
                    qs = slice(qlo + o, qlo + 512)
                    sp = ps.tile([128, 2, 512], F32, tag="sp", name="sp")
                    nc.tensor.matmul(sp[:, 0, o:512],
                                     khat2[0:64, kc],
                                     qhat2[m][0:64, qs],
                                     start=True, stop=True)
                    nc.tensor.matmul(sp[:, 1, o:512],
                                     khat2[64:128, kc],
                                     qhat2[m][64:128, qs],
                                     start=True, stop=True)
                    pbf = stage.tile([128, 2, 512], BF16, tag="pbf",
                                     bufs=6)
                    pbfs[kt] = pbf
                    nc.scalar.activation(pbf[:, :, o:512],
                                         sp[:, :, o:512], AF.Exp,
                                         bias=0.0, scale=SCALE)
                    if o or kt == 4 * qb:
                        # diag tile: zero the upper triangle (gpsimd: sbuf-
                        # only and cheap at 128 free elems; spares the DVE)
                        nc.gpsimd.tensor_mul(
                            pbf[:, 0, o:o + 128],
                            pbf[:, 0, o:o + 128], tri[:])
                        nc.gpsimd.tensor_mul(
                            pbf[:, 1, o:o + 128],
                            pbf[:, 1, o:o + 128], tri[:])

                def attnv(kt):
                    o = offs[kt]
                    pbf = pbfs[kt]
                    st_, sp_ = (kt == 0), (kt == nkt - 1)
                    nc.tensor.matmul(ctx_e[:, o:512], vp[:, kt, :],
                                     pbf[:, 0, o:512],
                                     start=st_, stop=sp_,
                                     skip_group_check=True)
                    nc.tensor.matmul(ctx_o[:, o:512], vp[:, kt, :],
                                     pbf[:, 1, o:512],
                                     start=st_, stop=sp_,
                                     skip_group_check=True)
                    pbfs[kt] = None

                # process kts in bursts of two: 2x scores back-to-back, then
                # 2x attnv — denser PE bursts keep the HAM clock warm
                scores(0)
                scores(1)
                for kt in range(2, nkt, 2):
                    scores(kt)
                    filler.drain(DRAINS[qb][0])
                    scores(kt + 1)
                    attnv(kt - 2)
                    attnv(kt - 1)
                    filler.drain(DRAINS[qb][1])
                attnv(nkt - 2)
                attnv(nkt - 1)
                if debug and qb == 0 and m == 0:
                    nc.vector.tensor_copy(out=dbg_ctxe[:, :], in_=ctx_e[:, :])
                    nc.vector.tensor_copy(out=dbg_ctxo[:, :], in_=ctx_o[:, :])

                # normalize: full-height evict (row 64 = l), recip at p64,
                # K=1 matmul broadcast, stt multiply.  No DMA in the chain.
                qs = slice(qlo, qlo + 512)
                last_pair = (qb == 3 and m == 1)

                def keep_warm():
                    # during the final norm chain the filler is dry; emit a
                    # junk matmul so HAM keeps the PE at 2.4GHz for oproj(3)
                    if last_pair:
                        warm_mm()
                    else:
                        filler.drain(1)

                ctxu = stage.tile([65, 2, 512], F32, tag="ctxu", bufs=2)
                nc.vector.tensor_copy(out=ctxu[:, 0, :], in_=ctx_e[:, :])
                nc.scalar.copy(out=ctxu[:, 1, :], in_=ctx_o[:, :])
                keep_warm()
                # move l rows to partition 0 (tiny dma, off critical path:
                # ctx psum is already freed by the ctxu eviction above)
                lsb = stage.tile([1, 2, 512], F32, tag="lsb", bufs=2)
                leng = nc.scalar if last_pair else nc.sync
                leng.dma_start(out=lsb[0:1, :, :], in_=ctxu[64:65, :, :])
                keep_warm()
                rr = stage.tile([1, 2, 512], F32, tag="rr", bufs=2)
                nc.vector.reciprocal_approx_fast(
                    rr[0:1, :, :], lsb[0:1, :, :])
                keep_warm()
                rr_bf = stage.tile([1, 2, 512], BF16, tag="rrbf", bufs=2)
                nc.vector.tensor_copy(out=rr_bf[0:1, :, :],
                                      in_=rr[0:1, :, :])
                keep_warm()
                rbp = [None, None]
                for h in range(2):
                    rbp[h] = py.tile([128, 512], F32, tag="yp",
                                     name=f"rbp{h}")
                    nc.tensor.matmul(rbp[h][0:64, :], ones64[0:1, :],
                                     rr_bf[0:1, h, :],
                                     start=True, stop=True)
                if debug and qb == 0 and m == 0:
                    for h in range(2):
                        nc.vector.tensor_copy(out=dbg_rrb[:, h, :],
                                              in_=rbp[h][0:64, :])
                nc.vector.scalar_tensor_tensor(
                    out=ctxn[m][0:64, qs],
                    in0=ctxu[0:64, 0, :], scalar=1.0,
                    in1=rbp[0][0:64, :], op0=MUL, op1=MUL)
                cn1 = stage.tile([64, 512], BF16, tag="cn1", bufs=2)
                nc.vector.scalar_tensor_tensor(
                    out=cn1[:], in0=ctxu[0:64, 1, :], scalar=1.0,
                    in1=rbp[1][0:64, :], op0=MUL, op1=MUL)
                nc.sync.dma_start(out=ctxn[m][64:128, qs], in_=cn1[:])

            # ================= schedule =================
            # startup: all proj matmuls first, rope tails after, so the PE
            # FIFO never head-of-line-blocks on the serial rope chains
            kv0 = unit_kv(0, early=True)
            q00 = unit_q(0, 0, early=True)
            for s in kv0[0:4]:      # pre + kv mms
                s()
            for s in q00[0:4]:      # q mms
                s()
            q00[4]()                # q evict (frees pq ring slot early)
            for s in kv0[4:]:       # kv evict/rope/kdup/vt/vp
                s()
            for s in q00[5:]:       # q rope
                s()

            filler.add_unit("q01", unit_q(0, 1))
            filler.add_units("g1", unit_kv(1), unit_q(1, 0), unit_q(1, 1))
            filler.add_units("g2", unit_kv(2), unit_q(2, 0), unit_q(2, 1))

            attn(0, 0)
            filler.ensure("q01")
            attn(0, 1)
            filler.add_unit("op0", unit_oproj(0))
            filler.add_units("g3", unit_kv(3), unit_q(3, 0), unit_q(3, 1))
            filler.ensure("g1")
            attn(1, 0)
            attn(1, 1)
            filler.add_unit("op1", unit_oproj(1))
            filler.ensure("g2")
            attn(2, 0)
            attn(2, 1)
            filler.add_unit("op2", unit_oproj(2))
            filler.ensure("g3")
            attn(3, 0)
            attn(3, 1)
            filler.drain_all()
            for s in unit_oproj(3, last=True):
                s()

            if debug:
                nc.sync.dma_start(out=khat_dump[:, :], in_=khat2[:, :])
                for m in range(2):
                    nc.sync.dma_start(out=qhat_dump[m][:, :],
                                      in_=qhat2[m][:, :])
                    nc.sync.dma_start(out=ctxn_dump[m][:, :],
                                      in_=ctxn[m][:, :])
                for kt in range(NKT):
                    nc.sync.dma_start(
                        out=vp_dump[:, kt * (HD + 1):(kt + 1) * (HD + 1)],
                        in_=vp[:, kt, :])
                nc.sync.dma_start(out=ctxe_dump[:, :], in_=dbg_ctxe[:, :])
                nc.sync.dma_start(out=ctxo_dump[:, :], in_=dbg_ctxo[:, :])
                for h in range(2):
                    nc.sync.dma_start(out=rrb_dump[:, h * 512:(h + 1) * 512],
                                      in_=dbg_rrb[:, h, :])

    nc.compile()
    return nc


def _host_constants():
    # rotation matrix (lhsT layout): rot = R @ qT with R[2i,2i+1]=-1, R[2i+1,2i]=1
    rt = np.zeros((128, 128), np.float32)
    i = np.arange(0, 128, 2)
    rt[i + 1, i] = -1.0     # lhsT[j, d] = R[d, j]
    rt[i, i + 1] = 1.0
    rt_bf = rt.astype(ml_dtypes.bfloat16)

    # tri[p, f] = 1 if f >= p else 0  (within-tile causal triangle)
    f = np.arange(128)[None, :]
    p = np.arange(128)[:, None]
    tri = (f >= p).astype(ml_dtypes.bfloat16)
    return rt_bf, tri


def _tile_inputs(x_b, sin, cos, mask_b, Wq_g, Wkv_g, Wo_g):
    """Host-side pre-tiling so every bulk load is one contiguous DMA."""
    bf = ml_dtypes.bfloat16
    xT = np.ascontiguousarray(x_b.T)                      # [C, T]
    xtl = np.ascontiguousarray(
        xT.reshape(8, 128, NT, 512).transpose(2, 1, 0, 3)).astype(bf)
    sinT = sin.T                                          # [64, T]
    cosT = cos.T
    sc = np.stack([np.concatenate([sinT, sinT], axis=0),
                   np.concatenate([cosT, cosT], axis=0)], axis=1)  # [128,2,T]
    sctl = np.ascontiguousarray(
        sc.reshape(128, 2, NT, 512).transpose(2, 0, 1, 3)).astype(bf)
    wqtl = np.ascontiguousarray(
        Wq_g.reshape(8, 128, 2, 128).transpose(2, 1, 0, 3)).astype(bf)
    wkvtl = np.ascontiguousarray(
        Wkv_g.reshape(8, 128, 128).transpose(1, 0, 2)).astype(bf)
    wotl = np.ascontiguousarray(
        Wo_g.reshape(2, 128, C).transpose(1, 0, 2)).astype(bf)
    return {
        "xtl": xtl, "sctl": sctl,
        "maskb": np.ascontiguousarray(mask_b.reshape(16, 128)),
        "wqtl": wqtl, "wkvtl": wkvtl, "wotl": wotl,
    }


def kernel(x, sin, cos, mask, Wq, Wk, Wv, Wo):
    global LAST_EXEC_NS
    if "nc" not in _CACHE:
        _CACHE["nc"] = _build()
    nc = _CACHE["nc"]

    x = np.asarray(x, np.float32)
    sin = np.asarray(sin, np.float32)
    cos = np.asarray(cos, np.float32)
    mask = np.asarray(mask, np.float32)
    Wq, Wk, Wv, Wo = (np.asarray(w, np.float32) for w in (Wq, Wk, Wv, Wo))
    rt_bf, tri = _host_constants()

    in_maps = []
    for core in range(NCORES):
        b, g = divmod(core, KVH)
        wkv = np.concatenate([Wk[:, g * HD:(g + 1) * HD],
                              Wv[:, g * HD:(g + 1) * HD]], axis=1)
        im = _tile_inputs(x[b], sin, cos, mask[b, 0],
                          np.ascontiguousarray(Wq[:, g * DG:(g + 1) * DG]),
                          np.ascontiguousarray(wkv),
                          np.ascontiguousarray(Wo[g * DG:(g + 1) * DG, :]))
        im["rt"] = rt_bf
        im["tri"] = tri
        in_maps.append(im)

    trace = os.environ.get("KERNEL_TRACE", "0") == "1"
    if trace:
        _install_trace_hook()
    res = run_bass_kernel_spmd(nc, in_maps, core_ids=list(range(NCORES)),
                               trace=trace)
    LAST_EXEC_NS = res.exec_time_ns

    y = np.zeros((B, T, C), np.float32)
    for core in range(NCORES):
        b = core // KVH
        ytl = np.asarray(res.results[core]["ytl"]).astype(np.float32)
        # ytl [NT, 8, 128, 512] -> yT [C, T] -> y [T, C]
        yT = ytl.transpose(1, 2, 0, 3).reshape(C, T)
        y[b] += yT.T
    return y
